# revision 12
# baseline (speedup 1.0000x reference)
"""3D Gaussian smoothing (sigma=2, ksize=13, 'same' zero padding) on 8 Trainium2 cores.

Strategy
--------
The Gaussian kernel W = g (x) g (x) g is separable, so the 13^3 dense conv
collapses into three 13-tap 1D convolutions (along D, H, W).  Each 1D conv is
executed on the TensorEngine as a multiplication with a small banded matrix.

Sharding: 8 cores = 2 batch x 4 depth-chunks of 48 output slices.  Each core
receives its depth chunk plus a 6-slice halo on both sides (zero padded at the
global edges), i.e. an input block of [60, 192, 192] fp32.

Per-core pipeline (all matmuls contract over the SBUF partition axis):
  A) D-conv:   load x as [(hs,d)=120 part, (hg,w) free] (h split even/odd so
               two h rows share each partition column group), multiply with a
               block-diagonal band matrix Bd [128, 96] -> psum[(hs,d')=96, .].
  S) shuffle:  SBUF->SBUF DMA rearrange into per-slice layout
               X_B[(hs,hg) part = h, (d', w) free]  (one true layout flip).
  B) H-conv:   per slice, lhsT = data [h, w-chunk] (stationary),
               rhs = band [h, h'] -> psum [w-chunk, h']  (conv + transpose).
  C) W-conv:   same trick again: lhsT = [w, h'-chunk], rhs = band [w, w']
               -> psum [h'-chunk, w']  (transposes back).
  O) store     y[d', h', w'].
"""

import os

import numpy as np

import ml_dtypes

import concourse.bass as bass
import concourse.mybir as mybir
import concourse.tile as tile
from concourse import bacc, bass_utils

# problem geometry (hardcoded per contract)
B = 2
D = H = W = 192
KS = 13
PAD = KS // 2  # 6
N_CORES = 8
DCHUNKS = 4
DP = D // DCHUNKS  # 48 output slices per core
DIN = DP + 2 * PAD  # 60 input slices per core
HS = 2
HG = H // HS  # 96

F32 = mybir.dt.float32
BF16 = mybir.dt.bfloat16
F32R = mybir.dt.float32r

_CACHE = {}


def _band_matrix(g, n_rows, n_cols, row_to_in, col_to_out):
    """M[r, c] = g[in - out + PAD] if |in - out| <= PAD else 0."""
    m = np.zeros((n_rows, n_cols), np.float32)
    for r in range(n_rows):
        hi = row_to_in(r)
        for c in range(n_cols):
            ho = col_to_out(c)
            k = hi - ho + PAD
            if 0 <= k < KS:
                m[r, c] = g[k]
    return m


def _host_constants(g):
    """Build the five band matrices from the 13-tap gaussian g (fp32)."""
    g = np.asarray(g, np.float64)

    # Phase A: Bd[(hs,d), (hs,d')], block diagonal over hs; d in [0,60), d' in [0,48).
    bd = np.zeros((128, HS * DP), np.float64)
    for hs in range(HS):
        for d in range(DIN):
            for dp in range(DP):
                k = d - dp  # x_local[dp + k], k in [0, 12]
                if 0 <= k < KS:
                    bd[hs * DIN + d, hs * DP + dp] = g[k]

    # Phase B: rows follow X_B1/X_B2 partition layout p = hs*64 + hg.
    def h_of_p1(p):
        hs, hg = divmod(p, 64)
        return 2 * hg + hs  # h in [0, 128)

    def h_of_p2(p):
        hs, hg0 = divmod(p, 64)
        return 2 * (hg0 + 32) + hs  # h in [64, 192)

    bb1 = _band_matrix(g, 128, 96, h_of_p1, lambda c: c)        # h' 0:96
    bb2 = _band_matrix(g, 128, 96, h_of_p2, lambda c: 96 + c)   # h' 96:192

    # Phase C: natural rows (psum partition order): Y_B1 w 0:128, Y_B2 w 64:192.
    bc1 = _band_matrix(g, 128, 96, lambda r: r, lambda c: c)         # w' 0:96
    bc2 = _band_matrix(g, 128, 96, lambda r: 64 + r, lambda c: 96 + c)  # w' 96:192

    return (bd.astype(np.float32), bb1, bb2, bc1, bc2)


def _build_program(sim_safe=False, bc_bf16=False, a_f32r=False):
    """Trace + compile the per-core Bass program (identical on all cores)."""
    nc = bacc.Bacc("TRN2", target_bir_lowering=False, debug=False)

    x = nc.dram_tensor("x", [DIN, H, W], F32, kind="ExternalInput").ap()
    bd_d = nc.dram_tensor("bd", [128, HS * DP], F32, kind="ExternalInput").ap()
    BDT = BF16 if bc_bf16 else F32
    bb1_d = nc.dram_tensor("bb1", [128, 96], BDT, kind="ExternalInput").ap()
    bb2_d = nc.dram_tensor("bb2", [128, 96], BDT, kind="ExternalInput").ap()
    bc1_d = nc.dram_tensor("bc1", [128, 96], BDT, kind="ExternalInput").ap()
    bc2_d = nc.dram_tensor("bc2", [128, 96], BDT, kind="ExternalInput").ap()
    y = nc.dram_tensor("y", [DP, H, W], F32, kind="ExternalOutput").ap()

    NLOAD = 8        # input DMA chunks (along hg)
    NSHUF = 4        # shuffle d'-groups
    SG = DP // NSHUF  # 12

    with tile.TileContext(nc) as tc:
        with tc.tile_pool(name="consts", bufs=1) as pconst, \
             tc.tile_pool(name="stage", bufs=1) as pstage, \
             tc.tile_pool(name="ps", bufs=4, space="PSUM") as pps:
            bd = pconst.tile([128, HS * DP], F32, name="bd_s")
            bb1 = pconst.tile([128, 96], BDT, name="bb1_s")
            bb2 = pconst.tile([128, 96], BDT, name="bb2_s")
            bc1 = pconst.tile([128, 96], BDT, name="bc1_s")
            bc2 = pconst.tile([128, 96], BDT, name="bc2_s")
            nc.sync.dma_start(bd[:], bd_d)
            nc.sync.dma_start(bb1[:], bb1_d)
            nc.sync.dma_start(bb2[:], bb2_d)
            nc.sync.dma_start(bc1[:], bc1_d)
            nc.sync.dma_start(bc2[:], bc2_d)

            # Two uniform 72KB/partition slots; phases ping-pong through
            # aliased views (Tile tracks region overlaps for ordering).
            SLOT0 = pstage.tile([128, HG * W], F32, tag="s0", name="SLOT0")
            SLOT1 = pstage.tile([128, HG * W], F32, tag="s1", name="SLOT1")

            # ------------- Phase A: D-conv -------------
            X_A = SLOT0.rearrange("p (a b) -> p a b", a=HG)
            nc.any.memset(X_A[96:128], 0.0)
            x_r = x.rearrange("d (hg hs) w -> d hs hg w", hs=HS).transpose([1, 0, 2, 3])
            hgb = HG // NLOAD
            for i in range(NLOAD):
                for hs in range(HS):
                    nc.sync.dma_start(
                        X_A[hs * DIN : (hs + 1) * DIN, i * hgb : (i + 1) * hgb, :],
                        x_r[hs, :, i * hgb : (i + 1) * hgb, :],
                    )

            if bc_bf16:
                Y_A = SLOT1.bitcast(BF16)[0 : HS * DP, 0 : HG * W].rearrange(
                    "p (a b) -> p a b", a=HG)
            else:
                Y_A = SLOT1[0 : HS * DP].rearrange("p (a b) -> p a b", a=HG)
            for i in range(HG // 2):
                ps = pps.tile([HS * DP, 2 * W], F32, tag="ps", name="psA")
                rhs_a = X_A[:, 2 * i : 2 * i + 2, :].rearrange("p a b -> p (a b)")
                lhs_a = bd[:]
                if a_f32r:
                    rhs_a = rhs_a.bitcast(F32R)
                    lhs_a = lhs_a.bitcast(F32R)
                nc.tensor.matmul(ps[:], lhs_a, rhs_a, start=True, stop=True)
                nc.scalar.copy(
                    Y_A[:, 2 * i : 2 * i + 2, :].rearrange("p a b -> p (a b)"), ps[:])

            # ------------- Shuffle: Y_A -> X_B (layout flip) -------------
            if bc_bf16:
                X_B = SLOT0.bitcast(BF16)[:, 0 : 2 * DP * W].rearrange(
                    "p (t d w) -> p t d w", t=2, d=DP)
            else:
                X_B = SLOT0.rearrange("p (t d w) -> p t d w", t=2, d=DP)
            X_B1, X_B2 = X_B[:, 0], X_B[:, 1]
            for hs in range(HS):
                for ti, hg0 in ((0, 0), (1, 32)):
                    for dp in range(DP):
                        row = hs * DP + dp
                        src = Y_A[row : row + 1, hg0 : hg0 + 64, :].rearrange(
                            "p a b -> p (a b)")                        # [1, 12288] contiguous
                        dst = X_B[hs * 64 : (hs + 1) * 64, ti, dp, :]  # [64 part, 192]
                        nc.sync.dma_start(dst, src)

            # ------------- Phase B: H-conv (+ transpose) -------------
            if bc_bf16:
                Y_B = SLOT1.bitcast(BF16)[:, 0 : 2 * DP * H].rearrange(
                    "p (t d w) -> p t d w", t=2, d=DP)
            else:
                Y_B = SLOT1.rearrange("p (t d w) -> p t d w", t=2, d=DP)
            Y_B1, Y_B2 = Y_B[:, 0], Y_B[:, 1]
            wsl = (slice(0, 128), slice(64, 192))
            for mc, ws in enumerate(wsl):
                ybt = (Y_B1, Y_B2)[mc]
                for i in range(DP // 2):
                    ps = pps.tile([128, 2 * H], F32, tag="ps", name="psB")
                    for j in range(2):
                        s = 2 * i + j
                        nc.tensor.matmul(ps[:, j * H : j * H + 96],
                                         X_B1[:, s, ws], bb1[:], start=True, stop=True)
                        nc.tensor.matmul(ps[:, j * H + 96 : (j + 1) * H],
                                         X_B2[:, s, ws], bb2[:], start=True, stop=True)
                    nc.vector.tensor_copy(
                        ybt[:, 2 * i : 2 * i + 2, :].rearrange("p a b -> p (a b)"), ps[:])

            # ------------- Phase C: W-conv (+ transpose back) -------------
            Y_C = SLOT0.rearrange("p (t d w) -> p t d w", t=2, d=DP)
            Y_C1, Y_C2 = Y_C[:, 0], Y_C[:, 1]
            for mc, hps in enumerate(wsl):
                yct = (Y_C1, Y_C2)[mc]
                for i in range(DP // 2):
                    ps = pps.tile([128, 2 * W], F32, tag="ps", name="psC")
                    for j in range(2):
                        s = 2 * i + j
                        nc.tensor.matmul(ps[:, j * W : j * W + 96],
                                         Y_B1[:, s, hps], bc1[:], start=True, stop=True)
                        nc.tensor.matmul(ps[:, j * W + 96 : (j + 1) * W],
                                         Y_B2[:, s, hps], bc2[:], start=True, stop=True)
                    nc.scalar.copy(
                        yct[:, 2 * i : 2 * i + 2, :].rearrange("p a b -> p (a b)"), ps[:])

            # ------------- Store -------------
            NOUT = 4
            og = DP // NOUT
            for i in range(NOUT):
                nc.sync.dma_start(
                    y[i * og : (i + 1) * og, 0:128, :].transpose([1, 0, 2]),
                    Y_C1[:, i * og : (i + 1) * og, :],
                )
                nc.sync.dma_start(
                    y[i * og : (i + 1) * og, 128:192, :].transpose([1, 0, 2]),
                    Y_C2[64:128, i * og : (i + 1) * og, :],
                )

    nc.compile()
    return nc


def _extract_g(Wk):
    """Recover the 13-tap 1D gaussian from the separable 3D kernel."""
    w3 = np.asarray(Wk, np.float64).reshape(KS, KS, KS)
    total = w3.sum()
    marg = w3.sum(axis=(1, 2))
    return marg / total ** (2.0 / 3.0)


def kernel(x, W):
    return _kernel_impl(np.asarray(x), np.asarray(W))


def _kernel_impl(x, Wk):
    assert x.shape == (B, 1, D, H, W), x.shape
    g = _extract_g(Wk)
    bd, bb1, bb2, bc1, bc2 = _host_constants(g)

    bc_bf16 = os.environ.get("GS_BF16", "1") == "1"
    a_f32r = os.environ.get("GS_F32R", "0") == "1"
    key = ("nc", bc_bf16, a_f32r)
    if key not in _CACHE:
        _CACHE[key] = _build_program(bc_bf16=bc_bf16, a_f32r=a_f32r)
    nc = _CACHE[key]
    if bc_bf16:
        bt = ml_dtypes.bfloat16
        bb1, bb2, bc1, bc2 = (a.astype(bt) for a in (bb1, bb2, bc1, bc2))

    xp = np.zeros((B, D + 2 * PAD, H, W), np.float32)
    xp[:, PAD : PAD + D] = x[:, 0]

    consts = {"bd": bd, "bb1": bb1, "bb2": bb2, "bc1": bc1, "bc2": bc2}
    in_maps = []
    for core in range(N_CORES):
        b, dc = divmod(core, DCHUNKS)
        shard = np.ascontiguousarray(xp[b, dc * DP : dc * DP + DIN])
        in_maps.append({"x": shard, **consts})

    res = bass_utils.run_bass_kernel_spmd(nc, in_maps, core_ids=list(range(N_CORES)))
    _CACHE["last_results"] = res

    out = np.empty((B, 1, D, H, W), np.float32)
    for core in range(N_CORES):
        b, dc = divmod(core, DCHUNKS)
        out[b, 0, dc * DP : (dc + 1) * DP] = res.results[core]["y"]
    return out
